# revision 1
# baseline (speedup 1.0000x reference)
"""Multi-head attention (B=2, S=2048, EMB=1024, H=16, hd=64) on 8 TRN2 cores.

Sharding: core c -> batch b = c//4, head-group g = c%4 (4 heads, 256 emb dims).
Per core (all matmuls in float32r: full-rate streaming, ~1e-4 rel err):
  A) Q^T = Wq_g @ x_b^T   [256, 2048]   (transposed layout, contraction on emb)
     K^T = Wk_g @ x_b^T   [256, 2048]
     V   = x_b @ Wv_g^T   [2048, 256]   (natural layout, +ones column per head)
  B) per head h: S^T[k,q] = K_h @ Q_h^T (16 k-tiles x [128, 2048] psum)
     P^T = exp(S^T/8) on ACT; U_aug[65, 2048] += [V_h|1].T @ P^T  (row 64 = softmax sums)
  C) r = 1/sums (DVE recip approx); broadcast r over 64 partitions via
     indicator matmul; O^T = U^T * r  (written over the Q^T buffer)
  D) y = O @ Wo_g^T partial [2048, 1024]; host sums the 4 head-group partials.
"""
import numpy as np

import concourse.bass as bass
import concourse.tile as tile
from concourse import bacc, mybir
from concourse.bass_utils import run_bass_kernel_spmd

import os

F32 = mybir.dt.float32
F32R = mybir.dt.float32r
BF16 = mybir.dt.bfloat16
FP16 = mybir.dt.float16
# matmul dtype: f32r (2 cyc/row, ~4e-4) | fp16 (1 cyc/row, ~1e-3) | bf16
MM_DT_NAME = os.environ.get("MM_DT", "fp16")
MM = {"f32r": F32R, "bf16": BF16, "fp16": FP16}[MM_DT_NAME]
IN_DT = {"f32r": F32, "bf16": BF16, "fp16": FP16}[MM_DT_NAME]
EXP = mybir.ActivationFunctionType.Exp
MULT = mybir.AluOpType.mult

EMB = 1024
S = 2048
B = 2
HG = 4           # heads per core
HD = 64
CHD = HG * HD    # 256 emb dims per core
ET = EMB // 128  # 8 e-tiles
NT = S // 128    # 16 s/k-tiles
QB = 512
NQB = S // QB    # 4

_NC = None


def _mm(ap):
    """View a dram input AP with the matmul dtype (bitcast only for f32r)."""
    return ap.bitcast(F32R) if MM == F32R else ap


def _build(dbg=False):
    nc = bacc.Bacc("TRN2", target_bir_lowering=False, debug=False)
    xq_t = nc.dram_tensor("xq_t", [EMB, S], IN_DT, kind="ExternalInput").ap()
    xk_t = nc.dram_tensor("xk_t", [EMB, S], IN_DT, kind="ExternalInput").ap()
    xv_t = nc.dram_tensor("xv_t", [EMB, S], IN_DT, kind="ExternalInput").ap()
    wq_t = nc.dram_tensor("wq_t", [EMB, CHD], IN_DT, kind="ExternalInput").ap()
    wk_t = nc.dram_tensor("wk_t", [EMB, CHD], IN_DT, kind="ExternalInput").ap()
    wv_t = nc.dram_tensor("wv_t", [EMB, CHD], IN_DT, kind="ExternalInput").ap()
    wo_t = nc.dram_tensor("wo_t", [CHD, EMB], IN_DT, kind="ExternalInput").ap()
    y = nc.dram_tensor("y", [S, EMB], F32, kind="ExternalOutput").ap()
    if dbg:
        dbg_qT = nc.dram_tensor("dbg_qT", [128, 2, S], F32, kind="ExternalOutput").ap()
        dbg_kT = nc.dram_tensor("dbg_kT", [128, 2, S], F32, kind="ExternalOutput").ap()
        dbg_v = nc.dram_tensor("dbg_v", [128, NT, HG * (HD + 1)], F32,
                               kind="ExternalOutput").ap()
        dbg_u = nc.dram_tensor("dbg_u", [HG, HD + 1, S], F32,
                               kind="ExternalOutput").ap()
        dbg_r = nc.dram_tensor("dbg_r", [HG, S], F32, kind="ExternalOutput").ap()
        dbg_oT = nc.dram_tensor("dbg_oT", [128, 2, S], F32, kind="ExternalOutput").ap()

    with tile.TileContext(nc) as tc:
        with tc.tile_pool(name="const", bufs=1) as cpool, \
             tc.tile_pool(name="wqk", bufs=2) as wpool, \
             tc.tile_pool(name="big", bufs=1) as big, \
             tc.tile_pool(name="usb", bufs=4) as usb, \
             tc.tile_pool(name="xp", bufs=8) as xp, \
             tc.tile_pool(name="pt", bufs=2) as ptp, \
             tc.tile_pool(name="yp", bufs=2) as ypool, \
             tc.tile_pool(name="rp", bufs=2) as rpool, \
             tc.tile_pool(name="rd", bufs=4, space="DRAM") as rdram:

            # ---- static weights (wo DMA deferred past phase A) ----
            wo_sb = cpool.tile([128, 2, EMB], MM, name="wo_sb")

            qT = big.tile([128, 2, S], MM, name="qT")     # later reused as O^T
            kT = big.tile([128, 2, S], MM, name="kT")
            v_sb = big.tile([128, NT, HG * (HD + 1)], MM, name="v_sb")
            if MM == F32R:
                nc.vector.memset(v_sb[:].bitcast(F32), 1.0)
            else:
                nc.vector.memset(v_sb[:], 1.0)     # ones cols survive

            # ---- phase A: projections ----
            warm0 = cpool.tile([128, QB], MM, name="warm0")
            nc.vector.memset(warm0[:], 1.0)
            with tc.tile_pool(name="psA", bufs=8, space="PSUM") as psA:
                # Q^T and K^T: out[m, q] accumulated over e; x-tile outer
                for name, xdram, wdram, dst in (
                        ("q", xq_t, wq_t, qT), ("k", xk_t, wk_t, kT)):
                    w_sb = wpool.tile([128, ET, CHD], MM, tag="w",
                                      name=f"w{name}_sb")
                    nc.sync.dma_start(
                        w_sb[:],
                        _mm(wdram).rearrange("(po pi) m -> pi po m", pi=128))
                    pss = [psA.tile([128, QB], F32, tag="ps", name=f"ps_{name}{i}")
                           for i in range(8)]
                    for e in range(ET):
                        x_t = xp.tile([128, S], MM, tag="x", name=f"x_{name}{e}")
                        nc.sync.dma_start(
                            x_t[:], _mm(xdram)[e * 128:(e + 1) * 128, :])
                        for m in range(2):
                            for qb in range(NQB):
                                nc.tensor.matmul(
                                    pss[m * NQB + qb][:],
                                    w_sb[:, e, m * 128:(m + 1) * 128],
                                    x_t[:, qb * QB:(qb + 1) * QB],
                                    start=(e == 0), stop=(e == ET - 1))
                    for m in range(2):
                        for qb in range(NQB):
                            cp = nc.scalar.copy if (m + qb) % 2 else \
                                nc.vector.tensor_copy
                            cp(dst[:, m, qb * QB:(qb + 1) * QB],
                               pss[m * NQB + qb][:])

                # V natural: s-outer; all 8 xv e-tiles stay resident (bf16)
                # so each s-tile owns one psum accumulation group.
                wv_sb = wpool.tile([128, ET, CHD], MM, tag="w", name="wv_sb")
                nc.sync.dma_start(
                    wv_sb[:],
                    _mm(wv_t).rearrange("(po pi) m -> pi po m", pi=128))
                xv_tiles = []
                for e in range(ET):
                    x_t = xp.tile([128, S], MM, tag="x", name=f"x_v{e}")
                    nc.sync.dma_start(
                        x_t[:], _mm(xv_t)[e * 128:(e + 1) * 128, :])
                    xv_tiles.append(x_t)
                # deferred weight loads ride behind the xv DMAs
                nc.sync.dma_start(
                    wo_sb[:], _mm(wo_t).rearrange("(ct p) n -> p ct n", p=128))
                for s in range(NT):
                    v_ps = psA.tile([128, CHD], F32, tag="ps", name=f"ps_v{s}")
                    for e in range(ET):
                        nc.tensor.matmul(
                            v_ps[:], xv_tiles[e][:, s * 128:(s + 1) * 128],
                            wv_sb[:, e, :],
                            start=(e == 0), stop=(e == ET - 1))
                    src = v_ps[:].rearrange("p (h d) -> p h d", d=HD)
                    dst = v_sb[:, s, :].rearrange("p (h d) -> p h d",
                                                  d=HD + 1)[:, :, 0:HD]
                    cp = nc.scalar.copy if s % 2 else nc.vector.tensor_copy
                    cp(dst, src)

            # ---- phase B: attention, head-PAIRS packed on PE ----
            # Heads 2mh (rows 0-63) and 2mh+1 (rows 64-127) issue scores
            # matmuls into different PE row-groups + different psum banks, so
            # they run concurrently. q is split in halves so both heads'
            # U accumulators fit PSUM ([65, 1024] = 2 banks each).
            u_list = [None] * HG
            with tc.tile_pool(name="psS", bufs=1, space="PSUM") as psS, \
                 tc.tile_pool(name="psU", bufs=1, space="PSUM") as psU:
                # PE warm-up: dense dummy matmuls so the HAM clock gate sits
                # at K=8/8 entering the latency-sensitive B phase.
                trash = psS.tile([128, S // 2], F32, tag="spsA", name="warm")
                for w in range(24):
                    nc.tensor.matmul(
                        trash[:, (w % 2) * QB:(w % 2 + 1) * QB],
                        v_sb[:, 0, 0:128],
                        v_sb[:, 0:2, 0:256],
                        start=True, stop=True)
                SH = S // 2
                for mh in range(2):
                    hA, hB = 2 * mh, 2 * mh + 1
                    uA = usb.tile([HD + 1, S], F32, tag="u", name=f"u{hA}")
                    uB = usb.tile([HD + 1, S], F32, tag="u", name=f"u{hB}")
                    u_list[hA], u_list[hB] = uA, uB
                    for qh in range(2):
                        qo = qh * SH
                        uaccA = psU.tile([HD + 1, SH], F32, tag="uaccA",
                                         name=f"uaccA{mh}_{qh}")
                        uaccB = psU.tile([HD + 1, SH], F32, tag="uaccB",
                                         name=f"uaccB{mh}_{qh}")
                        for t in range(NT):
                            spA = psS.tile([128, SH], F32, tag="spsA",
                                           name=f"spsA{mh}{qh}{t}")
                            spB = psS.tile([128, SH], F32, tag="spsB",
                                           name=f"spsB{mh}{qh}{t}")
                            for j in range(2):
                                for bp, sp in ((0, spA), (64, spB)):
                                    nc.tensor.matmul(
                                        sp[:, j * QB:(j + 1) * QB],
                                        kT[bp:bp + HD, mh,
                                           t * 128:(t + 1) * 128],
                                        qT[bp:bp + HD, mh,
                                           qo + j * QB:qo + (j + 1) * QB],
                                        start=True, stop=True)
                            pA = ptp.tile([128, SH], MM, tag="ptA",
                                          name=f"ptA{mh}{qh}{t}")
                            nc.scalar.activation(pA[:], spA[:], EXP, scale=0.125)
                            pB = ptp.tile([128, SH], MM, tag="ptB",
                                          name=f"ptB{mh}{qh}{t}")
                            nc.scalar.activation(pB[:], spB[:], EXP, scale=0.125)
                            for h2, uacc, p_t in ((hA, uaccA, pA),
                                                  (hB, uaccB, pB)):
                                for j in range(2):
                                    nc.tensor.matmul(
                                        uacc[:, j * QB:(j + 1) * QB],
                                        v_sb[:, t,
                                             h2 * (HD + 1):(h2 + 1) * (HD + 1)],
                                        p_t[:, j * QB:(j + 1) * QB],
                                        start=(t == 0), stop=(t == NT - 1))
                        nc.vector.tensor_copy(uA[:, qo:qo + SH], uaccA[:])
                        nc.scalar.copy(uB[:, qo:qo + SH], uaccB[:])
                    # softmax normalization for this pair: r = 1/sums, then
                    # broadcast r across 64 partitions via a DRAM-bounce DMA
                    # (stride-0 partition reads are legal from DRAM). O^T
                    # overwrites qT. Pair 0's work overlaps pair 1's B loop.
                    # gather the two sums rows to partition base 0 (custom
                    # DVE recip misbehaves on base-64 reads), one recip for
                    # the pair, then DRAM-bounce broadcast per head.
                    rp2 = rpool.tile([2, S], F32, tag="rh", name=f"rp2_{mh}")
                    nc.sync.dma_start(rp2[0:1, :], uA[HD:HD + 1, :])
                    nc.sync.dma_start(rp2[1:2, :], uB[HD:HD + 1, :])
                    r2 = rpool.tile([2, S], F32, tag="rh2", name=f"r2_{mh}")
                    nc.vector.reciprocal_approx_fast(out=r2[:], in_=rp2[:])
                    for h2, u_h in ((hA, uA), (hB, uB)):
                        bp2 = 64 * (h2 % 2)
                        rd = rdram.tile([1, S], F32, name=f"rd{h2}")
                        nc.sync.dma_start(rd[:], r2[h2 % 2:h2 % 2 + 1, :])
                        rb = rpool.tile([HD, S], F32, tag="rb", name=f"rb{h2}", bufs=3)
                        nc.sync.dma_start(rb[:], rd[:].to_broadcast([HD, S]))
                        for qh2 in range(2):
                            o2 = qh2 * SH
                            nc.vector.tensor_tensor(
                                qT[bp2:bp2 + HD, mh, o2:o2 + SH],
                                u_h[0:HD, o2:o2 + SH],
                                rb[:, o2:o2 + SH], MULT)

            if dbg:
                nc.sync.dma_start(dbg_qT, qT[:].bitcast(F32)) if MM == F32R else None
                nc.sync.dma_start(dbg_kT, kT[:].bitcast(F32)) if MM == F32R else None
                nc.sync.dma_start(dbg_v, v_sb[:].bitcast(F32)) if MM == F32R else None
                for h in range(HG):
                    nc.sync.dma_start(dbg_u[h], u_list[h][:])

            # ---- phase D: output projection (qT now holds O^T) ----
            with tc.tile_pool(name="psY", bufs=4, space="PSUM") as psY:
                # keep PE busy across the tail of the normalization chain
                tr1 = psY.tile([128, EMB], F32, tag="yps", name="warm1ps")
                for w in range(24):
                    nc.tensor.matmul(tr1[:, 0:QB], warm0[:, 0:128], warm0[:],
                                     start=True, stop=True)
                for s in range(NT):
                    y_ps = psY.tile([128, EMB], F32, tag="yps", name=f"yps{s}")
                    for nb in range(2):
                        for ct in range(2):
                            nc.tensor.matmul(
                                y_ps[:, nb * QB:(nb + 1) * QB],
                                qT[:, ct, s * 128:(s + 1) * 128],
                                wo_sb[:, ct, nb * QB:(nb + 1) * QB],
                                start=(ct == 0), stop=(ct == 1))
                    y_sb = ypool.tile([128, EMB], F32, tag="ysb",
                                      name=f"ysb{s}")
                    cp = nc.scalar.copy if s % 2 else nc.vector.tensor_copy
                    cp(y_sb[:], y_ps[:])
                    nc.sync.dma_start(y[s * 128:(s + 1) * 128, :], y_sb[:])

    nc.compile()
    return nc


def get_nc():
    global _NC
    if _NC is None:
        _NC = _build()
    return _NC


def make_in_maps(query, key, value, Wq, Wk, Wv, Wo):
    import ml_dtypes
    np_dt = {F32R: np.float32, BF16: ml_dtypes.bfloat16,
             FP16: np.float16}[MM]
    query = np.asarray(query, dtype=np.float32)
    key = np.asarray(key, dtype=np.float32)
    value = np.asarray(value, dtype=np.float32)
    Wq = np.asarray(Wq, dtype=np.float32)
    Wk = np.asarray(Wk, dtype=np.float32)
    Wv = np.asarray(Wv, dtype=np.float32)
    Wo = np.asarray(Wo, dtype=np.float32)
    xt = {(n, b): np.ascontiguousarray(x[b].T).astype(np_dt)
          for n, x in (("q", query), ("k", key), ("v", value))
          for b in range(B)}
    in_maps = []
    for c in range(8):
        b, g = divmod(c, 4)
        hs = slice(g * CHD, (g + 1) * CHD)
        in_maps.append({
            "xq_t": xt[("q", b)],
            "xk_t": xt[("k", b)],
            "xv_t": xt[("v", b)],
            "wq_t": np.ascontiguousarray(Wq[hs, :].T).astype(np_dt),
            "wk_t": np.ascontiguousarray(Wk[hs, :].T).astype(np_dt),
            "wv_t": np.ascontiguousarray(Wv[hs, :].T).astype(np_dt),
            "wo_t": np.ascontiguousarray(Wo[:, hs].T).astype(np_dt),
        })
    return in_maps


def gather(results):
    out = np.zeros((B, S, EMB), dtype=np.float32)
    for c in range(8):
        out[c // 4] += results[c]["y"]
    return out


def kernel(**inputs) -> np.ndarray:
    nc = get_nc()
    in_maps = make_in_maps(**inputs)
    res = run_bass_kernel_spmd(nc, in_maps, core_ids=list(range(8)))
    return gather(res.results)



# revision 5
# speedup vs baseline: 1.0559x; 1.0559x over previous
"""Multi-head attention (B=2, S=2048, EMB=1024, H=16, hd=64) on 8 TRN2 cores.

Sharding: core c -> batch b = c//4, head-group g = c%4 (4 heads, 256 emb dims).
Per core (fp16 matmuls, f32 psum):
  A) Q^T = Wq_g @ x_b^T  [256, 2048], K^T likewise (transposed layout);
     V = x_b @ Wv_g^T [2048, 256] natural layout with a ones column per head
     (row 64 of the AV accumulator then holds the softmax sums).
  B) per head-PAIR mh (heads 2mh, 2mh+1 packed on PE row halves), per
     q-window of 512, per k-tile t: S^T tile [128, 1024] (both heads) in
     psum; exp via ACT *or* DVE (int16 Schraudolph bit-trick, tunable
     fraction) -> fp16 P; AV matmuls accumulate U_aug [65, 512] per head.
  C) softmax: sums row -> SBUF-DMA bounce to partition 0 -> DVE fast recip
     -> gpsimd partition_broadcast -> DVE multiply writes O^T over qT.
  D) y partial per pair: O^T_mh @ Wo_mh -> fp16 y0/y1 outputs; pair 0's
     matmuls/evacs/DMAs are interleaved into pair 1's attention loop; host
     sums the 8 partials per batch.
"""
import os

import numpy as np

import concourse.bass as bass
import concourse.tile as tile
from concourse import bacc, mybir
from concourse.bass_utils import run_bass_kernel_spmd

F32 = mybir.dt.float32
F16 = mybir.dt.float16
I16 = mybir.dt.int16
EXP = mybir.ActivationFunctionType.Exp
MULT = mybir.AluOpType.mult
ADD = mybir.AluOpType.add

EMB = 1024
S = 2048
B = 2
HG = 4           # heads per core
HD = 64
CHD = HG * HD    # 256 emb dims per core
ET = EMB // 128  # 8 e-tiles
NT = S // 128    # 16 k-tiles
QB = 512
W = 512          # q-window for phase B
NW = S // W      # 4

# Schraudolph exp-on-DVE: i16 = f32_to_i16(s * SCH_A + SCH_B); bits as fp16.
# SCH_A folds the 1/8 temperature. SIGMA tunes the rounding/chord bias.
SCH_A = (2.0 ** 10) * 1.4426950408889634 / 8.0
SIGMA = float(os.environ.get("SCH_SIGMA", "-35.0"))
SCH_B = 15.0 * 1024.0 + SIGMA
# of each 16 k-tiles, how many go to the DVE (0..16)
DVE_N = int(os.environ.get("DVE_N", "5"))

_NC = None


def _dve_tile(t):
    # spread DVE tiles evenly over the 16 k-tiles
    return (t * DVE_N) % NT < DVE_N


def _build():
    nc = bacc.Bacc("TRN2", target_bir_lowering=False, debug=False)
    xq_t = nc.dram_tensor("xq_t", [EMB, S], F16, kind="ExternalInput").ap()
    xk_t = nc.dram_tensor("xk_t", [EMB, S], F16, kind="ExternalInput").ap()
    xv_t = nc.dram_tensor("xv_t", [EMB, S], F16, kind="ExternalInput").ap()
    wq_t = nc.dram_tensor("wq_t", [EMB, CHD], F16, kind="ExternalInput").ap()
    wk_t = nc.dram_tensor("wk_t", [EMB, CHD], F16, kind="ExternalInput").ap()
    wv_t = nc.dram_tensor("wv_t", [EMB, CHD], F16, kind="ExternalInput").ap()
    wo_t = nc.dram_tensor("wo_t", [CHD, EMB], F16, kind="ExternalInput").ap()
    y_out = [nc.dram_tensor(f"y{m}", [S, EMB], F16, kind="ExternalOutput").ap()
             for m in range(2)]

    with tile.TileContext(nc) as tc:
        with tc.tile_pool(name="const", bufs=1) as cpool, \
             tc.tile_pool(name="big", bufs=1) as big, \
             tc.tile_pool(name="usb", bufs=2) as usb, \
             tc.tile_pool(name="pt", bufs=3) as ptp, \
             tc.tile_pool(name="yp", bufs=3) as ypool:

            wo_sb = cpool.tile([128, 2, EMB], F16, name="wo_sb")

            qT = big.tile([128, 2, S], F16, name="qT")     # later holds O^T
            kT = big.tile([128, 2, S], F16, name="kT")
            v_sb = big.tile([128, NT, HG * (HD + 1)], F16, name="v_sb")
            nc.vector.memset(v_sb[:], 1.0)                 # ones cols survive

            # ---- phase A: projections ----
            with tc.tile_pool(name="wqk", bufs=2) as wpool, \
                 tc.tile_pool(name="xp", bufs=8) as xp, \
                 tc.tile_pool(name="psA", bufs=8, space="PSUM") as psA:
                for name, xdram, wdram, dst in (
                        ("q", xq_t, wq_t, qT), ("k", xk_t, wk_t, kT)):
                    w_sb = wpool.tile([128, ET, CHD], F16, tag="w",
                                      name=f"w{name}_sb")
                    nc.sync.dma_start(
                        w_sb[:], wdram.rearrange("(po pi) m -> pi po m", pi=128))
                    pss = [psA.tile([128, QB], F32, tag="ps", name=f"ps_{name}{i}")
                           for i in range(8)]
                    for e in range(ET):
                        x_t = xp.tile([128, S], F16, tag="x", name=f"x_{name}{e}")
                        nc.sync.dma_start(x_t[:], xdram[e * 128:(e + 1) * 128, :])
                        for m in range(2):
                            for qb in range(4):
                                nc.tensor.matmul(
                                    pss[m * 4 + qb][:],
                                    w_sb[:, e, m * 128:(m + 1) * 128],
                                    x_t[:, qb * QB:(qb + 1) * QB],
                                    start=(e == 0), stop=(e == ET - 1))
                    for m in range(2):
                        for qb in range(4):
                            cp = nc.scalar.copy if (m + qb) % 2 else \
                                nc.vector.tensor_copy
                            cp(dst[:, m, qb * QB:(qb + 1) * QB],
                               pss[m * 4 + qb][:])

                # V natural layout; all 8 xv e-tiles resident
                wv_sb = wpool.tile([128, ET, CHD], F16, tag="w", name="wv_sb")
                nc.sync.dma_start(
                    wv_sb[:], wv_t.rearrange("(po pi) m -> pi po m", pi=128))
                xv_tiles = []
                for e in range(ET):
                    x_t = xp.tile([128, S], F16, tag="x", name=f"x_v{e}")
                    nc.sync.dma_start(x_t[:], xv_t[e * 128:(e + 1) * 128, :])
                    xv_tiles.append(x_t)
                nc.sync.dma_start(
                    wo_sb[:], wo_t.rearrange("(ct p) n -> p ct n", p=128))
                for s in range(NT):
                    v_ps = psA.tile([128, CHD], F32, tag="ps", name=f"ps_v{s}")
                    for e in range(ET):
                        nc.tensor.matmul(
                            v_ps[:], xv_tiles[e][:, s * 128:(s + 1) * 128],
                            wv_sb[:, e, :],
                            start=(e == 0), stop=(e == ET - 1))
                    src = v_ps[:].rearrange("p (h d) -> p h d", d=HD)
                    dstv = v_sb[:, s, :].rearrange("p (h d) -> p h d",
                                                   d=HD + 1)[:, :, 0:HD]
                    cp = nc.scalar.copy if s % 2 else nc.vector.tensor_copy
                    cp(dstv, src)

            # ---- phases B + C + D ----
            # D work for pair 0 is interleaved into pair 1's attention loop.
            u_sb = {}   # (mh, head_in_pair) -> [65, S] f32 accumulators

            def emit_d_tile(m, s, y_ps):
                for nb in range(2):
                    nc.tensor.matmul(
                        y_ps[:, nb * QB:(nb + 1) * QB],
                        qT[:, m, s * 128:(s + 1) * 128],
                        wo_sb[:, m, nb * QB:(nb + 1) * QB],
                        start=True, stop=True)
                y_sb = ypool.tile([128, EMB], F16, tag="ysb", name=f"ysb{m}_{s}")
                cp = nc.scalar.copy if s % 2 else nc.vector.tensor_copy
                cp(y_sb[:], y_ps[:])
                nc.sync.dma_start(y_out[m][s * 128:(s + 1) * 128, :], y_sb[:])

            def emit_norm(mh):
                # softmax normalization for pair mh: O^T = U * (1/sums)
                for h2 in range(2):
                    u = u_sb[(mh, h2)]
                    sr = rpool.tile([1, S], F32, tag=f"sr{h2}",
                                    name=f"sr{mh}_{h2}")
                    nc.sync.dma_start(sr[:], u[HD:HD + 1, :])
                    rr = rpool.tile([1, S], F32, tag=f"rr{h2}",
                                    name=f"rr{mh}_{h2}")
                    nc.vector.reciprocal_approx_fast(out=rr[:], in_=sr[:])
                    rb = rpool.tile([HD, S], F32, tag=f"rb{h2}",
                                    name=f"rb{mh}_{h2}")
                    nc.gpsimd.partition_broadcast(rb[:], rr[:])
                    nc.vector.tensor_tensor(
                        qT[h2 * HD:(h2 + 1) * HD, mh, :],
                        u[0:HD, :], rb[:], MULT)

            with tc.tile_pool(name="rp", bufs=1) as rpool, \
                 tc.tile_pool(name="psB", bufs=2, space="PSUM") as psB, \
                 tc.tile_pool(name="psU", bufs=1, space="PSUM") as psU, \
                 tc.tile_pool(name="psY0", bufs=1, space="PSUM") as psY0:
                for mh in range(2):
                    hA, hB = 2 * mh, 2 * mh + 1
                    uA = usb.tile([HD + 1, S], F32, tag="uA", name=f"uA{mh}")
                    uB = usb.tile([HD + 1, S], F32, tag="uB", name=f"uB{mh}")
                    u_sb[(mh, 0)], u_sb[(mh, 1)] = uA, uB
                    # pair-0 D-work queue, drained inside pair 1's loop
                    d_queue = list(range(NT)) if mh == 1 else []
                    for qh in range(NW):
                        qo = qh * W
                        uaccA = psU.tile([HD + 1, W], F32, tag="uaccA",
                                         name=f"uaccA{mh}_{qh}")
                        uaccB = psU.tile([HD + 1, W], F32, tag="uaccB",
                                         name=f"uaccB{mh}_{qh}")
                        for t in range(NT):
                            sp = psB.tile([128, 2 * W], F32, tag="sp",
                                          name=f"sp{mh}{qh}{t}")
                            for bp, co in ((0, 0), (64, W)):
                                nc.tensor.matmul(
                                    sp[:, co:co + W],
                                    kT[bp:bp + HD, mh, t * 128:(t + 1) * 128],
                                    qT[bp:bp + HD, mh, qo:qo + W],
                                    start=True, stop=True)
                            pC = ptp.tile([128, 2 * W], F16, tag="pc",
                                          name=f"pc{mh}{qh}{t}")
                            if _dve_tile(t):
                                nc.vector.tensor_scalar(
                                    pC[:].bitcast(I16), sp[:],
                                    SCH_A, SCH_B, MULT, ADD)
                            else:
                                nc.scalar.activation(pC[:], sp[:], EXP,
                                                     scale=0.125)
                            for h2, uacc, co in ((hA, uaccA, 0),
                                                 (hB, uaccB, W)):
                                nc.tensor.matmul(
                                    uacc[:],
                                    v_sb[:, t, h2 * (HD + 1):
                                         (h2 + 1) * (HD + 1)],
                                    pC[:, co:co + W],
                                    start=(t == 0), stop=(t == NT - 1))
                            # drain one pair-0 output tile every 3rd t
                            if qh >= 1 and t % 3 == 2 and d_queue:
                                y_ps = psY0.tile([128, EMB], F32, tag="yps",
                                                 name=f"yps0_{d_queue[0]}")
                                emit_d_tile(0, d_queue.pop(0), y_ps)
                        cpA = nc.vector.tensor_copy if qh % 2 else \
                            nc.scalar.copy
                        cpB = nc.scalar.copy if qh % 2 else \
                            nc.vector.tensor_copy
                        cpA(uA[:, qo:qo + W], uaccA[:])
                        cpB(uB[:, qo:qo + W], uaccB[:])
                    emit_norm(mh)
                    if mh == 1:
                        for s in d_queue:   # leftovers
                            y_ps = psY0.tile([128, EMB], F32, tag="yps",
                                             name=f"yps0L_{s}")
                            emit_d_tile(0, s, y_ps)

            # ---- phase D tail: pair 1's output projection ----
            with tc.tile_pool(name="psY1", bufs=3, space="PSUM") as psY1:
                # keep the PE warm across the pair-1 normalization chain
                trash = psY1.tile([128, EMB], F32, tag="yps", name="warm")
                for wi in range(12):
                    nc.tensor.matmul(trash[:, 0:QB], v_sb[:, 0, 0:128],
                                     v_sb[:, 0:2, 0:256], start=True, stop=True)
                for s in range(NT):
                    y_ps = psY1.tile([128, EMB], F32, tag="yps", name=f"yps1_{s}")
                    emit_d_tile(1, s, y_ps)

    nc.compile()
    return nc


def get_nc():
    global _NC
    if _NC is None:
        _NC = _build()
    return _NC


def make_in_maps(query, key, value, Wq, Wk, Wv, Wo):
    query = np.asarray(query, dtype=np.float32)
    key = np.asarray(key, dtype=np.float32)
    value = np.asarray(value, dtype=np.float32)
    Wq = np.asarray(Wq, dtype=np.float32)
    Wk = np.asarray(Wk, dtype=np.float32)
    Wv = np.asarray(Wv, dtype=np.float32)
    Wo = np.asarray(Wo, dtype=np.float32)
    xt = {(n, b): np.ascontiguousarray(x[b].T).astype(np.float16)
          for n, x in (("q", query), ("k", key), ("v", value))
          for b in range(B)}
    in_maps = []
    for c in range(8):
        b, g = divmod(c, 4)
        hs = slice(g * CHD, (g + 1) * CHD)
        in_maps.append({
            "xq_t": xt[("q", b)],
            "xk_t": xt[("k", b)],
            "xv_t": xt[("v", b)],
            "wq_t": np.ascontiguousarray(Wq[hs, :].T).astype(np.float16),
            "wk_t": np.ascontiguousarray(Wk[hs, :].T).astype(np.float16),
            "wv_t": np.ascontiguousarray(Wv[hs, :].T).astype(np.float16),
            "wo_t": np.ascontiguousarray(Wo[:, hs].T).astype(np.float16),
        })
    return in_maps


def gather(results):
    out = np.zeros((B, S, EMB), dtype=np.float32)
    for c in range(8):
        out[c // 4] += results[c]["y0"].astype(np.float32)
        out[c // 4] += results[c]["y1"].astype(np.float32)
    return out


def kernel(**inputs) -> np.ndarray:
    nc = get_nc()
    in_maps = make_in_maps(**inputs)
    res = run_bass_kernel_spmd(nc, in_maps, core_ids=list(range(8)))
    return gather(res.results)
